# revision 20
# baseline (speedup 1.0000x reference)
"""Distributed Trainium2 Bass kernel for nn_Attention (GQA attention + LoRA + RoPE).

Sharding: tensor-parallel over heads across 8 NeuronCores.
  - core c owns Q heads 4c..4c+3 and KV head c (GQA group).
  - wq/wk/wv column-sharded; wo COLUMN-sharded (each core computes a
    512-column slice of the output over the full 4096 contraction, fed by an
    AllGather of all cores' per-head attention outputs).
  - LoRA is folded into wq/wv on the host (x@wq + (x@A)@B == x@(wq + A@B)).
  - 1/sqrt(HD) folded into wq.
  - RoPE pair permutation folded into wq/wk column order: within each head the
    even dims come first, odd dims second, so on-device RoPE is plain
    elementwise math on partition halves.

Everything the device computes is bf16-in/f32-accumulate.

v2 performance notes (vs the 494us baseline):
  - xt streamed in 4-ktile chunks (512KB DMAs) on the sync queue; weights
    streamed in big chunks on the scalar queue, so quarter 0 isn't serialized
    behind 4 small DMAs per k-tile on one queue.
  - causal attention narrowed to the lower triangle at 128-col granularity:
    QK / exp / PV / denominator matmuls only touch [r:512] of each sq chunk,
    affine_select only the 128-wide diagonal span.
  - wo reuses wq's SBUF (loaded after proj quarter 3's last wq read).
  - AllGather outputs read back in 2-ktile chunks on the idle sync queue.
  - gather(1) triggered before wo_batch(0); final evictions split between
    scalar and vector engines.
"""

import sys
import types

import numpy as np
import ml_dtypes

import concourse.bass as bass
from concourse import bacc
import concourse.mybir as mybir
import concourse.tile as tile
from concourse.bass_utils import run_bass_kernel_spmd
from concourse.masks import make_identity


def _ensure_axon_hooks():
    """run_bass_kernel_spmd(trace=True) imports antenv.axon_hooks, which some
    images lack; install a no-op shim so a BASS_TRACE env var can't crash us."""
    try:
        import antenv
    except ImportError:
        return
    if "antenv.axon_hooks" in sys.modules:
        return
    try:
        from antenv import axon_hooks  # noqa: F401
        return
    except ImportError:
        pass
    mod = types.ModuleType("antenv.axon_hooks")
    mod._hook = None
    mod.set_axon_ntff_profile_hook = lambda h: setattr(mod, "_hook", h)
    mod.get_axon_ntff_profile_hook = lambda: mod._hook
    sys.modules["antenv.axon_hooks"] = mod
    antenv.axon_hooks = mod


_ensure_axon_hooks()

B, S, D = 2, 1024, 4096
H, KVH, HD = 32, 8, 128
NCORES = 8
HPC = H // NCORES            # 4 q heads per core
QCOLS = HPC * HD             # 512
T = B * S                    # 2048
P = 128
KT = D // P                  # 32 k tiles
NQ = 4                       # token quarters (512 tokens each)
QW = T // NQ                 # 512
SQC = 2                      # sq chunks per batch
STB = S // P                 # 8 st blocks per batch
XTC = 4                      # k-tiles per xt DMA chunk
AGC = 2                      # k-tiles per allgather-readback DMA chunk

FP32 = mybir.dt.float32
BF16 = mybir.dt.bfloat16
EXP = mybir.ActivationFunctionType.Exp

_COMPILED = {}
LAST_RESULTS = None


def _st_info(variant, sqc):
    """st blocks contributing to sq chunk sqc, as (st, r, sel):
    r = first needed column within the 512-wide chunk (0 for full width),
    sel = start of the 128-wide diagonal span needing triangular zeroing
    (None if the block is fully below the diagonal / no mask)."""
    out = []
    for st in range(STB):
        if variant == "causal":
            rd = st * P - sqc * QW
            if rd >= QW:
                continue  # fully masked
            if rd >= 0:
                out.append((st, rd, rd))
            else:
                out.append((st, 0, None))
        else:
            out.append((st, 0, None))
    return out


def _build(variant):
    nc = bacc.Bacc(None)

    # xt packed quarter-major: [:, qx, k, :] is per-partition contiguous 4KB
    # per 4-ktile chunk, so xt chunk DMAs run at large-descriptor efficiency.
    xt_e = nc.declare_dram_parameter("xt", [P, NQ, KT, QW], BF16, isOutput=False)
    wq_e = nc.declare_dram_parameter("wq", [P, KT, QCOLS], BF16, isOutput=False)
    wk_e = nc.declare_dram_parameter("wk", [P, KT, HD], BF16, isOutput=False)
    wv_e = nc.declare_dram_parameter("wv", [P, KT, HD], BF16, isOutput=False)
    wo_e = nc.declare_dram_parameter("wo", [P, KT, QCOLS], BF16, isOutput=False)
    # cos: [c; c] duplicated halves.  sin: [s; -s] (negated bottom half).
    cos_e = nc.declare_dram_parameter("cos", [P, T], BF16, isOutput=False)
    sin_e = nc.declare_dram_parameter("sin", [P, T], BF16, isOutput=False)
    if variant == "general":
        mk_e = nc.declare_dram_parameter("mk", [P, STB, S], BF16, isOutput=False)
    out_e = nc.declare_dram_parameter("out", [QCOLS, T], BF16, isOutput=True)

    with tile.TileContext(nc) as tc:
        with (
            tc.tile_pool(name="wpool", bufs=1) as wpool,
            tc.tile_pool(name="cst", bufs=1) as cst,
            tc.tile_pool(name="persist", bufs=1) as persist,
            tc.tile_pool(name="xt", bufs=8) as xtp,
            tc.tile_pool(name="ev", bufs=5) as evp,
            tc.tile_pool(name="rt", bufs=4) as rtp,
            tc.tile_pool(name="probs", bufs=20) as prp,
            tc.tile_pool(name="misc", bufs=3) as mip,
            tc.tile_pool(name="ag", bufs=4) as agp,
            tc.tile_pool(name="ow", bufs=8) as owp,
            tc.tile_pool(name="ps", bufs=8, space="PSUM") as psp,
            tc.tile_pool(name="dram", bufs=1, space="DRAM") as dram,
        ):
            # ---- resident weights / constants ----
            # wq_sb doubles as wo storage: wo is DMA'd over it after proj
            # quarter 3's last wq read (Tile WAR tracking orders this).
            wq_sb = wpool.tile([P, KT, QCOLS], BF16, name="wq_sb")
            wk_sb = wpool.tile([P, KT, HD], BF16, name="wk_sb")
            wv_sb = wpool.tile([P, KT, HD], BF16, name="wv_sb")
            cos_sb = wpool.tile([P, T], BF16, name="cos_sb")
            sin_sb = wpool.tile([P, T], BF16, name="sin_sb")
            if variant == "general":
                mk_sb = wpool.tile([P, STB, S], BF16, name="mk_sb")

            ident = cst.tile([P, P], BF16, name="ident")
            make_identity(nc, ident)
            ones_sq = cst.tile([P, P], BF16, name="ones_sq")
            nc.vector.memset(ones_sq[:], 1.0)

            # ---- persistent activations ----
            q_rot = [[persist.tile([P, S], BF16, name=f"q{h}_{b}")
                      for b in range(B)] for h in range(HPC)]
            k_rot = [persist.tile([P, S], BF16, name=f"k{b}") for b in range(B)]
            v_sb = [persist.tile([P, STB, P], BF16, name=f"v{b}") for b in range(B)]
            attn = [[persist.tile([P, S], BF16, name=f"attn{h}_{b}")
                     for b in range(B)] for h in range(HPC)]

            ag_in = [dram.tile([HPC * P, S], BF16, name=f"agin{b}") for b in range(B)]
            ag_out = [dram.tile([H * P, S], BF16, addr_space="Shared",
                                name=f"agout{b}") for b in range(B)]

            def rope(dst, dst_off, src_bf, qoff):
                """RoPE on split layout (a=0:64, b=64:128).
                p1 = [a*c; b*c];  p2 = [a*s; -b*s];  swap halves of p2;
                dst = p1 + p2sw = [a*c - b*s; a*s + b*c]."""
                c = cos_sb[:, qoff:qoff + QW]
                s = sin_sb[:, qoff:qoff + QW]
                p1 = rtp.tile([P, QW], BF16, name="p1")
                p2 = rtp.tile([P, QW], BF16, name="p2")
                p2sw = rtp.tile([P, QW], BF16, name="p2sw")
                nc.vector.tensor_mul(p1[:], src_bf[:], c)
                nc.vector.tensor_mul(p2[:], src_bf[:], s)
                nc.vector.tensor_copy(p2sw[0:64, :], p2[64:128, :])
                nc.vector.tensor_copy(p2sw[64:128, :], p2[0:64, :])
                nc.vector.tensor_add(dst[:, dst_off:dst_off + QW], p1[:], p2sw[:])

            def load_weights():
                """Stream all projection weights + rope tables on the scalar
                (HWDGE) queue in big chunks, first-needed k-tiles first, so the
                sync queue only carries the xt stream.  The rope-table slices
                (needed only at each quarter's eviction, ~50us deadlines) are
                slotted mid-stream to keep them out of the startup burst."""
                dma = nc.scalar.dma_start

                def tables(qx):
                    toff = qx * QW
                    dma(cos_sb[:, toff:toff + QW], cos_e[:, toff:toff + QW])
                    dma(sin_sb[:, toff:toff + QW], sin_e[:, toff:toff + QW])

                # tiny first chunks so k=0 matmuls start ASAP
                dma(wk_sb[:, 0:2, :], wk_e[:, 0:2, :])
                dma(wv_sb[:, 0:2, :], wv_e[:, 0:2, :])
                dma(wq_sb[:, 0:2, :], wq_e[:, 0:2, :])
                dma(wk_sb[:, 2:8, :], wk_e[:, 2:8, :])
                dma(wv_sb[:, 2:8, :], wv_e[:, 2:8, :])
                for c in range(1, 5):
                    dma(wq_sb[:, 2 * c:2 * c + 2, :], wq_e[:, 2 * c:2 * c + 2, :])
                dma(wk_sb[:, 8:KT, :], wk_e[:, 8:KT, :])
                dma(wv_sb[:, 8:KT, :], wv_e[:, 8:KT, :])
                tables(0)
                for c in range(5, KT // 2):
                    dma(wq_sb[:, 2 * c:2 * c + 2, :], wq_e[:, 2 * c:2 * c + 2, :])
                    if c == 8:
                        tables(1)
                tables(2)
                tables(3)
                if variant == "general":
                    dma(mk_sb[:], mk_e[:])

            def wo_load():
                """Overwrite wq_sb with wo (WAR-ordered after the last wq read,
                i.e. streams during attention_batch(1)).  On the scalar queue so
                the sync queue only carries agt readback for wo_batch(0)."""
                for c in range(4):
                    nc.scalar.dma_start(wq_sb[:, 8 * c:8 * c + 8, :],
                                        wo_e[:, 8 * c:8 * c + 8, :])

            def proj_quarter(qx):
                b, boff = qx // 2, (qx % 2) * QW
                toff = qx * QW
                # psums: k, v, q0..q3
                psums = [psp.tile([P, QW], FP32, name="ps", tag="ps")
                         for _ in range(6)]

                def mm(mb, k, xt):
                    if mb == 0:
                        w = wk_sb[:, k, :]
                    elif mb == 1:
                        w = wv_sb[:, k, :]
                    else:
                        w = wq_sb[:, k, (mb - 2) * P:(mb - 1) * P]
                    nc.tensor.matmul(psums[mb][:], w, xt,
                                     start=(k == 0), stop=(k == KT - 1))

                chunks = [1] + [3] + [XTC] * ((KT - XTC) // XTC) if qx == 0 \
                    else [XTC] * (KT // XTC)
                k0 = 0
                for ci, cw in enumerate(chunks):
                    xt4 = xtp.tile([P, XTC, QW], BF16, name="xt4")
                    nc.sync.dma_start(xt4[:, 0:cw, :],
                                      xt_e[:, qx, k0:k0 + cw, :])
                    if ci < len(chunks) - 1:
                        for j in range(cw):
                            for mb in range(6):
                                mm(mb, k0 + j, xt4[:, j, :])
                    else:
                        # last chunk mb-major, V then K then Q heads: each
                        # accumulation stops a few us early so its eviction +
                        # rope/transpose overlaps the remaining matmuls and the
                        # following attention chunk starts without a gap.
                        for mb in (1, 0, 2, 3, 4, 5):
                            for j in range(cw):
                                mm(mb, k0 + j, xt4[:, j, :])
                    k0 += cw
                # scalar queue order: ve, ke, qe0..3, v_sb copies — so the k/q
                # ropes (DVE) start as early as possible; the v_sb copies only
                # gate the PV matmuls which run several us into attention.
                ve = evp.tile([P, QW], BF16, name="ve", tag="qe")
                nc.scalar.copy(ve[:], psums[1][:])
                tps = []
                for i in range(QW // P):
                    tp = psp.tile([P, P], BF16, name="tp", tag="ps")
                    nc.tensor.transpose(tp[:], ve[:, i * P:(i + 1) * P], ident[:])
                    tps.append(tp)
                ke = evp.tile([P, QW], BF16, name="ke", tag="qe")
                nc.scalar.copy(ke[:], psums[0][:])
                rope(k_rot[b], boff, ke, toff)
                for h in range(HPC):
                    qe = evp.tile([P, QW], BF16, name="qe", tag="qe")
                    nc.scalar.copy(qe[:], psums[2 + h][:])
                    rope(q_rot[h][b], boff, qe, toff)
                for i, tp in enumerate(tps):
                    st = (qx % 2) * 4 + i
                    nc.scalar.copy(v_sb[b][:, st, :], tp[:])

            def attention_batch(b, sqcs=tuple(range(SQC))):
                for sqc in sqcs:
                    sq0 = sqc * QW
                    stl = _st_info(variant, sqc)
                    n = len(stl)
                    for h in range(HPC):
                        # scores^T + exp, narrowed to [r:QW] per st block
                        prtiles = []
                        for st, r, sel in stl:
                            pss = psp.tile([P, QW], FP32, name="pss", tag="ps")
                            preload = variant == "general"
                            if preload:
                                nc.tensor.matmul(pss[:], ident[:],
                                                 mk_sb[:, st, sq0:sq0 + QW],
                                                 start=True, stop=False)
                            nc.tensor.matmul(
                                pss[:, r:QW], k_rot[b][:, st * P:(st + 1) * P],
                                q_rot[h][b][:, sq0 + r:sq0 + QW],
                                start=(not preload), stop=True)
                            pr = prp.tile([P, QW], BF16, name="pr", tag="pr")
                            nc.scalar.activation(pr[:, r:QW], pss[:, r:QW], EXP)
                            if sel is not None and variant == "causal":
                                # zero probs in the diagonal 128-wide span where
                                # st*128+p > sq0+q (future keys); columns beyond
                                # sel+P are fully below the diagonal.
                                nc.gpsimd.affine_select(
                                    out=pr[:, sel:sel + P],
                                    in_=pr[:, sel:sel + P],
                                    compare_op=mybir.AluOpType.is_ge,
                                    fill=0.0,
                                    base=0,
                                    channel_multiplier=-1,
                                    pattern=[[1, P]])
                            prtiles.append((pr, r))
                        # PV and denominator (denom broadcast to all partitions)
                        pso = psp.tile([P, QW], FP32, name="pso", tag="ps")
                        psdb = psp.tile([P, QW], FP32, name="psdb", tag="ps")
                        for i, (st, r, sel) in enumerate(stl):
                            pr = prtiles[i][0]
                            nc.tensor.matmul(pso[:, r:QW], v_sb[b][:, st, :],
                                             pr[:, r:QW],
                                             start=(i == 0), stop=(i == n - 1))
                            nc.tensor.matmul(psdb[:, r:QW], ones_sq[:],
                                             pr[:, r:QW],
                                             start=(i == 0), stop=(i == n - 1))
                        rb = mip.tile([P, QW], FP32, name="rb")
                        nc.vector.reciprocal_approx_fast(rb[:], psdb[:])
                        nc.vector.tensor_mul(attn[h][b][:, sq0:sq0 + QW],
                                             pso[:], rb[:])
                        if sqc == SQC - 1:
                            # ship this head to the gather bounce buffer ASAP
                            nc.gpsimd.dma_start(
                                ag_in[b][h * P:(h + 1) * P, :], attn[h][b][:])

            def gather_batch(b):
                nc.gpsimd.collective_compute(
                    "AllGather", mybir.AluOpType.bypass,
                    ins=[ag_in[b][:].opt()],
                    outs=[ag_out[b][:].opt()],
                    replica_groups=[list(range(NCORES))],
                )

            def wo_batch(b):
                ag_r = ag_out[b].rearrange("(k p) t -> p k t", p=P)
                psw = [[psp.tile([P, QW], FP32, name="psw", tag="ps")
                        for _ in range(SQC)] for _ in range(4)]
                nchk = KT // AGC
                for kc in range(nchk):
                    agt = agp.tile([P, AGC, S], BF16, name="agt")
                    nc.sync.dma_start(agt[:], ag_r[:, kc * AGC:(kc + 1) * AGC, :])
                    if kc < nchk - 1:
                        for j in range(AGC):
                            k = kc * AGC + j
                            for mb in range(4):
                                w = wq_sb[:, k, mb * P:(mb + 1) * P]
                                for nch in range(SQC):
                                    nc.tensor.matmul(
                                        psw[mb][nch][:], w,
                                        agt[:, j, nch * QW:(nch + 1) * QW],
                                        start=(k == 0), stop=False)
                    else:
                        # last chunk mb-major so early mb groups stop a few us
                        # before the end and their evictions + out DMAs overlap
                        # the remaining matmuls.
                        for mb in range(4):
                            for nch in range(SQC):
                                for j in range(AGC):
                                    k = kc * AGC + j
                                    w = wq_sb[:, k, mb * P:(mb + 1) * P]
                                    nc.tensor.matmul(
                                        psw[mb][nch][:], w,
                                        agt[:, j, nch * QW:(nch + 1) * QW],
                                        start=False, stop=(k == KT - 1))
                for mb in range(4):
                    for nch in range(SQC):
                        ow = owp.tile([P, QW], BF16, name="ow")
                        if (mb * SQC + nch) % 2 == 0:
                            nc.scalar.copy(ow[:], psw[mb][nch][:])
                        else:
                            nc.vector.tensor_copy(ow[:], psw[mb][nch][:])
                        nc.sync.dma_start(
                            out_e[mb * P:(mb + 1) * P,
                                  b * S + nch * QW:b * S + (nch + 1) * QW],
                            ow[:])

            # ---- timeline ----
            # attention sq-chunks interleave between proj quarters: chunk s0 of
            # batch b only needs that batch's first token quarter. This fires
            # gather(0) earlier and gives the xt/weight streams HBM-quiet
            # windows (attention phases do no HBM traffic) to get ahead.
            load_weights()
            proj_quarter(0)
            attention_batch(0, (0,))
            proj_quarter(1)
            attention_batch(0, (1,))
            gather_batch(0)
            proj_quarter(2)
            attention_batch(1, (0,))
            proj_quarter(3)
            wo_load()
            attention_batch(1, (1,))
            gather_batch(1)
            wo_batch(0)
            wo_batch(1)

    nc.compile()
    return nc


def _get_compiled(variant):
    if variant not in _COMPILED:
        _COMPILED[variant] = _build(variant)
    return _COMPILED[variant]


def _detect_variant(mask2d):
    if not np.any(mask2d):
        return "nomask"
    tril = np.tril(mask2d)
    if not np.any(tril):
        iu = np.triu_indices(S, 1)
        if np.all(mask2d[iu] <= -1e8):
            return "causal"
    return "general"


def _pack_kt(w):
    """[R*128, N] -> [128, R, N] so that [:, k, :] is rows k*128..k*128+127."""
    return np.ascontiguousarray(w.reshape(w.shape[0] // P, P, -1).transpose(1, 0, 2))


def kernel(x, wq, wk, wv, wo, lora_q_a, lora_q_b, lora_v_a, lora_v_b,
           freqs_cos, freqs_sin, mask, start_pos=0, **_):
    global LAST_RESULTS
    bf = ml_dtypes.bfloat16
    x = np.asarray(x, np.float32)
    wq = np.asarray(wq, np.float32)
    wk = np.asarray(wk, np.float32)
    wv = np.asarray(wv, np.float32)
    wo = np.asarray(wo, np.float32)
    lora_q_a = np.asarray(lora_q_a, np.float32)
    lora_q_b = np.asarray(lora_q_b, np.float32)
    lora_v_a = np.asarray(lora_v_a, np.float32)
    lora_v_b = np.asarray(lora_v_b, np.float32)
    cos = np.asarray(freqs_cos, np.float32)
    sin = np.asarray(freqs_sin, np.float32)
    mask2d = np.asarray(mask, np.float32).reshape(S, S)

    variant = _detect_variant(mask2d)
    nc = _get_compiled(variant)

    # fold LoRA + scale; permute rope pairs (evens then odds within each head)
    wq_eff = (wq + lora_q_a @ lora_q_b) * np.float32(1.0 / np.sqrt(HD))
    wv_eff = wv + lora_v_a @ lora_v_b
    perm = np.concatenate([np.arange(0, HD, 2), np.arange(1, HD, 2)])
    qperm = (np.arange(H)[:, None] * HD + perm[None, :]).reshape(-1)
    kperm = (np.arange(KVH)[:, None] * HD + perm[None, :]).reshape(-1)
    wq_eff = wq_eff[:, qperm]
    wk_p = wk[:, kperm]

    xt = np.ascontiguousarray(x.reshape(T, D).T)        # [4096, 2048]
    # [128, KT, T] -> quarter-major [128, NQ, KT, QW] (contiguous per chunk)
    xt_p = np.ascontiguousarray(
        _pack_kt(xt).reshape(P, KT, NQ, QW).transpose(0, 2, 1, 3)).astype(bf)
    c64 = np.tile(cos.T, (1, B))                        # [64, 2048]
    s64 = np.tile(sin.T, (1, B))
    cosT = np.concatenate([c64, c64], axis=0).astype(bf)   # [c; c]
    sinT = np.concatenate([s64, -s64], axis=0).astype(bf)  # [s; -s]

    if variant == "general":
        maskT = np.ascontiguousarray(mask2d.T)          # [st, sq]
        mk = _pack_kt(maskT).astype(bf)                 # [128, 8, 1024]
    else:
        mk = None

    in_maps = []
    for c in range(NCORES):
        im = {
            "xt": xt_p,
            "wq": _pack_kt(wq_eff[:, c * QCOLS:(c + 1) * QCOLS]).astype(bf),
            "wk": _pack_kt(wk_p[:, c * HD:(c + 1) * HD]).astype(bf),
            "wv": _pack_kt(wv_eff[:, c * HD:(c + 1) * HD]).astype(bf),
            "wo": _pack_kt(wo[:, c * QCOLS:(c + 1) * QCOLS]).astype(bf),
            "cos": cosT,
            "sin": sinT,
        }
        if mk is not None:
            im["mk"] = mk
        in_maps.append(im)

    res = run_bass_kernel_spmd(nc, in_maps, core_ids=list(range(NCORES)))
    LAST_RESULTS = res
    outT = np.concatenate([res.results[c]["out"] for c in range(NCORES)], axis=0)
    return np.ascontiguousarray(outT.T).reshape(B, S, D).astype(np.float32)


# revision 21
# speedup vs baseline: 1.1318x; 1.1318x over previous
"""Distributed Trainium2 Bass kernel for nn_Attention (GQA attention + LoRA + RoPE).

Sharding: tensor-parallel over heads across 8 NeuronCores.
  - core c owns Q heads 4c..4c+3 and KV head c (GQA group).
  - wq/wk/wv column-sharded; wo COLUMN-sharded (each core computes a
    512-column slice of the output over the full 4096 contraction, fed by an
    AllGather of all cores' per-head attention outputs).
  - LoRA is folded into wq/wv on the host (x@wq + (x@A)@B == x@(wq + A@B)).
  - 1/sqrt(HD) folded into wq.
  - RoPE pair permutation folded into wq/wk column order: within each head the
    even dims come first, odd dims second, so on-device RoPE is plain
    elementwise math on partition halves.

Everything the device computes is bf16-in/f32-accumulate.

v2 performance notes (vs the 494us baseline):
  - xt streamed in 4-ktile chunks (512KB DMAs) on the sync queue; weights
    streamed in big chunks on the scalar queue, so quarter 0 isn't serialized
    behind 4 small DMAs per k-tile on one queue.
  - causal attention narrowed to the lower triangle at 128-col granularity:
    QK / exp / PV / denominator matmuls only touch [r:512] of each sq chunk,
    affine_select only the 128-wide diagonal span.
  - wo reuses wq's SBUF (loaded after proj quarter 3's last wq read).
  - AllGather outputs read back in 2-ktile chunks on the idle sync queue.
  - gather(1) triggered before wo_batch(0); final evictions split between
    scalar and vector engines.
"""

import sys
import types

import numpy as np
import ml_dtypes

import concourse.bass as bass
from concourse import bacc
import concourse.mybir as mybir
import concourse.tile as tile
from concourse.bass_utils import run_bass_kernel_spmd
from concourse.masks import make_identity


def _ensure_axon_hooks():
    """run_bass_kernel_spmd(trace=True) imports antenv.axon_hooks, which some
    images lack; install a no-op shim so a BASS_TRACE env var can't crash us."""
    try:
        import antenv
    except ImportError:
        return
    if "antenv.axon_hooks" in sys.modules:
        return
    try:
        from antenv import axon_hooks  # noqa: F401
        return
    except ImportError:
        pass
    mod = types.ModuleType("antenv.axon_hooks")
    mod._hook = None
    mod.set_axon_ntff_profile_hook = lambda h: setattr(mod, "_hook", h)
    mod.get_axon_ntff_profile_hook = lambda: mod._hook
    sys.modules["antenv.axon_hooks"] = mod
    antenv.axon_hooks = mod


_ensure_axon_hooks()

B, S, D = 2, 1024, 4096
H, KVH, HD = 32, 8, 128
NCORES = 8
HPC = H // NCORES            # 4 q heads per core
QCOLS = HPC * HD             # 512
T = B * S                    # 2048
P = 128
KT = D // P                  # 32 k tiles
NQ = 4                       # token quarters (512 tokens each)
QW = T // NQ                 # 512
SQC = 2                      # sq chunks per batch
STB = S // P                 # 8 st blocks per batch
XTC = 4                      # k-tiles per xt DMA chunk
AGC = 2                      # k-tiles per allgather-readback DMA chunk

FP32 = mybir.dt.float32
BF16 = mybir.dt.bfloat16
EXP = mybir.ActivationFunctionType.Exp

_COMPILED = {}
LAST_RESULTS = None


def _st_info(variant, sqc):
    """st blocks contributing to sq chunk sqc, as (st, r, sel):
    r = first needed column within the 512-wide chunk (0 for full width),
    sel = start of the 128-wide diagonal span needing triangular zeroing
    (None if the block is fully below the diagonal / no mask)."""
    out = []
    for st in range(STB):
        if variant == "causal":
            rd = st * P - sqc * QW
            if rd >= QW:
                continue  # fully masked
            if rd >= 0:
                out.append((st, rd, rd))
            else:
                out.append((st, 0, None))
        else:
            out.append((st, 0, None))
    return out


def _build(variant):
    nc = bacc.Bacc(None)

    # xt packed quarter-major: [:, qx, k, :] is per-partition contiguous 4KB
    # per 4-ktile chunk, so xt chunk DMAs run at large-descriptor efficiency.
    xt_e = nc.declare_dram_parameter("xt", [P, NQ, KT, QW], BF16, isOutput=False)
    wq_e = nc.declare_dram_parameter("wq", [P, KT, QCOLS], BF16, isOutput=False)
    wk_e = nc.declare_dram_parameter("wk", [P, KT, HD], BF16, isOutput=False)
    wv_e = nc.declare_dram_parameter("wv", [P, KT, HD], BF16, isOutput=False)
    wo_e = nc.declare_dram_parameter("wo", [P, KT, QCOLS], BF16, isOutput=False)
    # cos: [c; c] duplicated halves.  sin: [s; -s] (negated bottom half).
    cos_e = nc.declare_dram_parameter("cos", [P, T], BF16, isOutput=False)
    sin_e = nc.declare_dram_parameter("sin", [P, T], BF16, isOutput=False)
    if variant == "general":
        mk_e = nc.declare_dram_parameter("mk", [P, STB, S], BF16, isOutput=False)
    out_e = nc.declare_dram_parameter("out", [QCOLS, T], BF16, isOutput=True)

    with tile.TileContext(nc) as tc:
        with (
            tc.tile_pool(name="wpool", bufs=1) as wpool,
            tc.tile_pool(name="cst", bufs=1) as cst,
            tc.tile_pool(name="persist", bufs=1) as persist,
            tc.tile_pool(name="xt", bufs=10) as xtp,
            tc.tile_pool(name="ev", bufs=4) as evp,
            tc.tile_pool(name="rt", bufs=3) as rtp,
            tc.tile_pool(name="probs", bufs=18) as prp,
            tc.tile_pool(name="misc", bufs=2) as mip,
            tc.tile_pool(name="ag", bufs=4) as agp,
            tc.tile_pool(name="ow", bufs=8) as owp,
            tc.tile_pool(name="ps", bufs=8, space="PSUM") as psp,
            tc.tile_pool(name="dram", bufs=1, space="DRAM") as dram,
        ):
            # ---- resident weights / constants ----
            # wq_sb doubles as wo storage: wo is DMA'd over it after proj
            # quarter 3's last wq read (Tile WAR tracking orders this).
            wq_sb = wpool.tile([P, KT, QCOLS], BF16, name="wq_sb")
            wk_sb = wpool.tile([P, KT, HD], BF16, name="wk_sb")
            wv_sb = wpool.tile([P, KT, HD], BF16, name="wv_sb")
            cos_sb = wpool.tile([P, T], BF16, name="cos_sb")
            sin_sb = wpool.tile([P, T], BF16, name="sin_sb")
            if variant == "general":
                mk_sb = wpool.tile([P, STB, S], BF16, name="mk_sb")

            ident = cst.tile([P, P], BF16, name="ident")
            make_identity(nc, ident)
            ones_sq = cst.tile([P, P], BF16, name="ones_sq")
            nc.vector.memset(ones_sq[:], 1.0)

            # ---- persistent activations ----
            q_rot = [[persist.tile([P, S], BF16, name=f"q{h}_{b}")
                      for b in range(B)] for h in range(HPC)]
            k_rot = [persist.tile([P, S], BF16, name=f"k{b}") for b in range(B)]
            v_sb = [persist.tile([P, STB, P], BF16, name=f"v{b}") for b in range(B)]
            attn = [[persist.tile([P, S], BF16, name=f"attn{h}_{b}")
                     for b in range(B)] for h in range(HPC)]

            ag_in = [dram.tile([HPC * P, S], BF16, name=f"agin{b}") for b in range(B)]
            ag_out = [dram.tile([H * P, S], BF16, addr_space="Shared",
                                name=f"agout{b}") for b in range(B)]

            def rope(dst, dst_off, src_bf, qoff):
                """RoPE on split layout (a=0:64, b=64:128).
                p1 = [a*c; b*c];  p2 = [a*s; -b*s];  swap halves of p2;
                dst = p1 + p2sw = [a*c - b*s; a*s + b*c]."""
                c = cos_sb[:, qoff:qoff + QW]
                s = sin_sb[:, qoff:qoff + QW]
                p1 = rtp.tile([P, QW], BF16, name="p1")
                p2 = rtp.tile([P, QW], BF16, name="p2")
                p2sw = rtp.tile([P, QW], BF16, name="p2sw")
                nc.vector.tensor_mul(p1[:], src_bf[:], c)
                nc.vector.tensor_mul(p2[:], src_bf[:], s)
                nc.vector.tensor_copy(p2sw[0:64, :], p2[64:128, :])
                nc.vector.tensor_copy(p2sw[64:128, :], p2[0:64, :])
                nc.vector.tensor_add(dst[:, dst_off:dst_off + QW], p1[:], p2sw[:])

            def load_weights():
                """Stream all projection weights + rope tables on the scalar
                (HWDGE) queue in big chunks, first-needed k-tiles first, so the
                sync queue only carries the xt stream.  The rope-table slices
                (needed only at each quarter's eviction, ~50us deadlines) are
                slotted mid-stream to keep them out of the startup burst."""
                dma = nc.scalar.dma_start

                def tables(qx):
                    toff = qx * QW
                    dma(cos_sb[:, toff:toff + QW], cos_e[:, toff:toff + QW])
                    dma(sin_sb[:, toff:toff + QW], sin_e[:, toff:toff + QW])

                # tiny first chunks so k=0 matmuls start ASAP
                dma(wk_sb[:, 0:2, :], wk_e[:, 0:2, :])
                dma(wv_sb[:, 0:2, :], wv_e[:, 0:2, :])
                dma(wq_sb[:, 0:2, :], wq_e[:, 0:2, :])
                dma(wk_sb[:, 2:8, :], wk_e[:, 2:8, :])
                dma(wv_sb[:, 2:8, :], wv_e[:, 2:8, :])
                for c in range(1, 5):
                    dma(wq_sb[:, 2 * c:2 * c + 2, :], wq_e[:, 2 * c:2 * c + 2, :])
                dma(wk_sb[:, 8:KT, :], wk_e[:, 8:KT, :])
                dma(wv_sb[:, 8:KT, :], wv_e[:, 8:KT, :])
                tables(0)
                for c in range(5, KT // 2):
                    dma(wq_sb[:, 2 * c:2 * c + 2, :], wq_e[:, 2 * c:2 * c + 2, :])
                    if c == 8:
                        tables(1)
                tables(2)
                tables(3)
                if variant == "general":
                    dma(mk_sb[:], mk_e[:])

            def wo_load():
                """Overwrite wq_sb with wo (WAR-ordered after the last wq read,
                i.e. streams during attention_batch(1)).  On the scalar queue so
                the sync queue only carries agt readback for wo_batch(0)."""
                for c in range(4):
                    nc.scalar.dma_start(wq_sb[:, 8 * c:8 * c + 8, :],
                                        wo_e[:, 8 * c:8 * c + 8, :])

            def proj_quarter(qx):
                b, boff = qx // 2, (qx % 2) * QW
                toff = qx * QW
                # psums: k, v, q0..q3
                psums = [psp.tile([P, QW], FP32, name="ps", tag="ps")
                         for _ in range(6)]

                def mm(mb, k, xt):
                    if mb == 0:
                        w = wk_sb[:, k, :]
                    elif mb == 1:
                        w = wv_sb[:, k, :]
                    else:
                        w = wq_sb[:, k, (mb - 2) * P:(mb - 1) * P]
                    nc.tensor.matmul(psums[mb][:], w, xt,
                                     start=(k == 0), stop=(k == KT - 1))

                chunks = [1] + [3] + [XTC] * ((KT - XTC) // XTC) if qx == 0 \
                    else [XTC] * (KT // XTC)
                k0 = 0
                for ci, cw in enumerate(chunks):
                    xt4 = xtp.tile([P, XTC, QW], BF16, name="xt4")
                    nc.sync.dma_start(xt4[:, 0:cw, :],
                                      xt_e[:, qx, k0:k0 + cw, :])
                    if ci < len(chunks) - 1:
                        for j in range(cw):
                            for mb in range(6):
                                mm(mb, k0 + j, xt4[:, j, :])
                    else:
                        # last chunk mb-major, V then K then Q heads: each
                        # accumulation stops a few us early so its eviction +
                        # rope/transpose overlaps the remaining matmuls and the
                        # following attention chunk starts without a gap.
                        for mb in (1, 0, 2, 3, 4, 5):
                            for j in range(cw):
                                mm(mb, k0 + j, xt4[:, j, :])
                    k0 += cw
                # scalar queue order: ve, ke, qe0..3, v_sb copies — so the k/q
                # ropes (DVE) start as early as possible; the v_sb copies only
                # gate the PV matmuls which run several us into attention.
                ve = evp.tile([P, QW], BF16, name="ve", tag="qe")
                nc.scalar.copy(ve[:], psums[1][:])
                tps = []
                for i in range(QW // P):
                    tp = psp.tile([P, P], BF16, name="tp", tag="ps")
                    nc.tensor.transpose(tp[:], ve[:, i * P:(i + 1) * P], ident[:])
                    tps.append(tp)
                ke = evp.tile([P, QW], BF16, name="ke", tag="qe")
                nc.scalar.copy(ke[:], psums[0][:])
                rope(k_rot[b], boff, ke, toff)
                for h in range(HPC):
                    qe = evp.tile([P, QW], BF16, name="qe", tag="qe")
                    nc.scalar.copy(qe[:], psums[2 + h][:])
                    rope(q_rot[h][b], boff, qe, toff)
                for i, tp in enumerate(tps):
                    st = (qx % 2) * 4 + i
                    nc.scalar.copy(v_sb[b][:, st, :], tp[:])

            def attention_batch(b, sqcs=tuple(range(SQC))):
                for sqc in sqcs:
                    sq0 = sqc * QW
                    stl = _st_info(variant, sqc)
                    n = len(stl)
                    for h in range(HPC):
                        # scores^T + exp, narrowed to [r:QW] per st block
                        prtiles = []
                        for st, r, sel in stl:
                            pss = psp.tile([P, QW], FP32, name="pss", tag="ps")
                            preload = variant == "general"
                            if preload:
                                nc.tensor.matmul(pss[:], ident[:],
                                                 mk_sb[:, st, sq0:sq0 + QW],
                                                 start=True, stop=False)
                            nc.tensor.matmul(
                                pss[:, r:QW], k_rot[b][:, st * P:(st + 1) * P],
                                q_rot[h][b][:, sq0 + r:sq0 + QW],
                                start=(not preload), stop=True)
                            pr = prp.tile([P, QW], BF16, name="pr", tag="pr")
                            nc.scalar.activation(pr[:, r:QW], pss[:, r:QW], EXP)
                            if sel is not None and variant == "causal":
                                # zero probs in the diagonal 128-wide span where
                                # st*128+p > sq0+q (future keys); columns beyond
                                # sel+P are fully below the diagonal.
                                nc.gpsimd.affine_select(
                                    out=pr[:, sel:sel + P],
                                    in_=pr[:, sel:sel + P],
                                    compare_op=mybir.AluOpType.is_ge,
                                    fill=0.0,
                                    base=0,
                                    channel_multiplier=-1,
                                    pattern=[[1, P]])
                            prtiles.append((pr, r))
                        # PV and denominator (denom broadcast to all partitions)
                        pso = psp.tile([P, QW], FP32, name="pso", tag="ps")
                        psdb = psp.tile([P, QW], FP32, name="psdb", tag="ps")
                        for i, (st, r, sel) in enumerate(stl):
                            pr = prtiles[i][0]
                            nc.tensor.matmul(pso[:, r:QW], v_sb[b][:, st, :],
                                             pr[:, r:QW],
                                             start=(i == 0), stop=(i == n - 1))
                            nc.tensor.matmul(psdb[:, r:QW], ones_sq[:],
                                             pr[:, r:QW],
                                             start=(i == 0), stop=(i == n - 1))
                        rb = mip.tile([P, QW], FP32, name="rb")
                        nc.vector.reciprocal_approx_fast(rb[:], psdb[:])
                        nc.vector.tensor_mul(attn[h][b][:, sq0:sq0 + QW],
                                             pso[:], rb[:])
                        if sqc == SQC - 1:
                            # ship this head to the gather bounce buffer ASAP
                            nc.gpsimd.dma_start(
                                ag_in[b][h * P:(h + 1) * P, :], attn[h][b][:])

            def gather_batch(b):
                nc.gpsimd.collective_compute(
                    "AllGather", mybir.AluOpType.bypass,
                    ins=[ag_in[b][:].opt()],
                    outs=[ag_out[b][:].opt()],
                    replica_groups=[list(range(NCORES))],
                )

            def wo_batch(b):
                ag_r = ag_out[b].rearrange("(k p) t -> p k t", p=P)
                psw = [[psp.tile([P, QW], FP32, name="psw", tag="ps")
                        for _ in range(SQC)] for _ in range(4)]
                nchk = KT // AGC
                for kc in range(nchk):
                    agt = agp.tile([P, AGC, S], BF16, name="agt")
                    nc.sync.dma_start(agt[:], ag_r[:, kc * AGC:(kc + 1) * AGC, :])
                    if kc < nchk - 1:
                        for j in range(AGC):
                            k = kc * AGC + j
                            for mb in range(4):
                                w = wq_sb[:, k, mb * P:(mb + 1) * P]
                                for nch in range(SQC):
                                    nc.tensor.matmul(
                                        psw[mb][nch][:], w,
                                        agt[:, j, nch * QW:(nch + 1) * QW],
                                        start=(k == 0), stop=False)
                    else:
                        # last chunk mb-major so early mb groups stop a few us
                        # before the end and their evictions + out DMAs overlap
                        # the remaining matmuls.
                        for mb in range(4):
                            for nch in range(SQC):
                                for j in range(AGC):
                                    k = kc * AGC + j
                                    w = wq_sb[:, k, mb * P:(mb + 1) * P]
                                    nc.tensor.matmul(
                                        psw[mb][nch][:], w,
                                        agt[:, j, nch * QW:(nch + 1) * QW],
                                        start=False, stop=(k == KT - 1))
                for mb in range(4):
                    for nch in range(SQC):
                        ow = owp.tile([P, QW], BF16, name="ow")
                        if (mb * SQC + nch) % 2 == 0:
                            nc.scalar.copy(ow[:], psw[mb][nch][:])
                            dma = nc.scalar.dma_start
                        else:
                            nc.vector.tensor_copy(ow[:], psw[mb][nch][:])
                            dma = nc.sync.dma_start
                        dma(out_e[mb * P:(mb + 1) * P,
                                  b * S + nch * QW:b * S + (nch + 1) * QW],
                            ow[:])

            # ---- timeline ----
            # attention sq-chunks interleave between proj quarters: chunk s0 of
            # batch b only needs that batch's first token quarter. This fires
            # gather(0) earlier and gives the xt/weight streams HBM-quiet
            # windows (attention phases do no HBM traffic) to get ahead.
            load_weights()
            proj_quarter(0)
            attention_batch(0, (0,))
            proj_quarter(1)
            attention_batch(0, (1,))
            gather_batch(0)
            proj_quarter(2)
            attention_batch(1, (0,))
            proj_quarter(3)
            wo_load()
            attention_batch(1, (1,))
            gather_batch(1)
            wo_batch(0)
            wo_batch(1)

    nc.compile()
    return nc


def _get_compiled(variant):
    if variant not in _COMPILED:
        _COMPILED[variant] = _build(variant)
    return _COMPILED[variant]


def _detect_variant(mask2d):
    if not np.any(mask2d):
        return "nomask"
    tril = np.tril(mask2d)
    if not np.any(tril):
        iu = np.triu_indices(S, 1)
        if np.all(mask2d[iu] <= -1e8):
            return "causal"
    return "general"


def _pack_kt(w):
    """[R*128, N] -> [128, R, N] so that [:, k, :] is rows k*128..k*128+127."""
    return np.ascontiguousarray(w.reshape(w.shape[0] // P, P, -1).transpose(1, 0, 2))


def kernel(x, wq, wk, wv, wo, lora_q_a, lora_q_b, lora_v_a, lora_v_b,
           freqs_cos, freqs_sin, mask, start_pos=0, **_):
    global LAST_RESULTS
    bf = ml_dtypes.bfloat16
    x = np.asarray(x, np.float32)
    wq = np.asarray(wq, np.float32)
    wk = np.asarray(wk, np.float32)
    wv = np.asarray(wv, np.float32)
    wo = np.asarray(wo, np.float32)
    lora_q_a = np.asarray(lora_q_a, np.float32)
    lora_q_b = np.asarray(lora_q_b, np.float32)
    lora_v_a = np.asarray(lora_v_a, np.float32)
    lora_v_b = np.asarray(lora_v_b, np.float32)
    cos = np.asarray(freqs_cos, np.float32)
    sin = np.asarray(freqs_sin, np.float32)
    mask2d = np.asarray(mask, np.float32).reshape(S, S)

    variant = _detect_variant(mask2d)
    nc = _get_compiled(variant)

    # fold LoRA + scale; permute rope pairs (evens then odds within each head)
    wq_eff = (wq + lora_q_a @ lora_q_b) * np.float32(1.0 / np.sqrt(HD))
    wv_eff = wv + lora_v_a @ lora_v_b
    perm = np.concatenate([np.arange(0, HD, 2), np.arange(1, HD, 2)])
    qperm = (np.arange(H)[:, None] * HD + perm[None, :]).reshape(-1)
    kperm = (np.arange(KVH)[:, None] * HD + perm[None, :]).reshape(-1)
    wq_eff = wq_eff[:, qperm]
    wk_p = wk[:, kperm]

    xt = np.ascontiguousarray(x.reshape(T, D).T)        # [4096, 2048]
    # [128, KT, T] -> quarter-major [128, NQ, KT, QW] (contiguous per chunk)
    xt_p = np.ascontiguousarray(
        _pack_kt(xt).reshape(P, KT, NQ, QW).transpose(0, 2, 1, 3)).astype(bf)
    c64 = np.tile(cos.T, (1, B))                        # [64, 2048]
    s64 = np.tile(sin.T, (1, B))
    cosT = np.concatenate([c64, c64], axis=0).astype(bf)   # [c; c]
    sinT = np.concatenate([s64, -s64], axis=0).astype(bf)  # [s; -s]

    if variant == "general":
        maskT = np.ascontiguousarray(mask2d.T)          # [st, sq]
        mk = _pack_kt(maskT).astype(bf)                 # [128, 8, 1024]
    else:
        mk = None

    in_maps = []
    for c in range(NCORES):
        im = {
            "xt": xt_p,
            "wq": _pack_kt(wq_eff[:, c * QCOLS:(c + 1) * QCOLS]).astype(bf),
            "wk": _pack_kt(wk_p[:, c * HD:(c + 1) * HD]).astype(bf),
            "wv": _pack_kt(wv_eff[:, c * HD:(c + 1) * HD]).astype(bf),
            "wo": _pack_kt(wo[:, c * QCOLS:(c + 1) * QCOLS]).astype(bf),
            "cos": cosT,
            "sin": sinT,
        }
        if mk is not None:
            im["mk"] = mk
        in_maps.append(im)

    res = run_bass_kernel_spmd(nc, in_maps, core_ids=list(range(NCORES)))
    LAST_RESULTS = res
    outT = np.concatenate([res.results[c]["out"] for c in range(NCORES)], axis=0)
    return np.ascontiguousarray(outT.T).reshape(B, S, D).astype(np.float32)


# revision 27
# speedup vs baseline: 1.1399x; 1.0071x over previous
"""Distributed Trainium2 Bass kernel for nn_Attention (GQA attention + LoRA + RoPE).

Sharding: tensor-parallel over heads across 8 NeuronCores.
  - core c owns Q heads 4c..4c+3 and KV head c (GQA group).
  - wq/wk/wv column-sharded; wo COLUMN-sharded (each core computes a
    512-column slice of the output over the full 4096 contraction, fed by an
    AllGather of all cores' per-head attention outputs).
  - LoRA is folded into wq/wv on the host (x@wq + (x@A)@B == x@(wq + A@B)).
  - 1/sqrt(HD) folded into wq.
  - RoPE pair permutation folded into wq/wk column order: within each head the
    even dims come first, odd dims second, so on-device RoPE is plain
    elementwise math on partition halves.

Everything the device computes is bf16-in/f32-accumulate.

v2 performance notes (vs the 494us baseline):
  - xt streamed in 4-ktile chunks (512KB DMAs) on the sync queue; weights
    streamed in big chunks on the scalar queue, so quarter 0 isn't serialized
    behind 4 small DMAs per k-tile on one queue.
  - causal attention narrowed to the lower triangle at 128-col granularity:
    QK / exp / PV / denominator matmuls only touch [r:512] of each sq chunk,
    affine_select only the 128-wide diagonal span.
  - wo reuses wq's SBUF (loaded after proj quarter 3's last wq read).
  - AllGather outputs read back in 2-ktile chunks on the idle sync queue.
  - gather(1) triggered before wo_batch(0); final evictions split between
    scalar and vector engines.
"""

import sys
import types

import numpy as np
import ml_dtypes

import concourse.bass as bass
from concourse import bacc
import concourse.mybir as mybir
import concourse.tile as tile
from concourse.bass_utils import run_bass_kernel_spmd
from concourse.masks import make_identity


def _ensure_axon_hooks():
    """run_bass_kernel_spmd(trace=True) imports antenv.axon_hooks, which some
    images lack; install a no-op shim so a BASS_TRACE env var can't crash us."""
    try:
        import antenv
    except ImportError:
        return
    if "antenv.axon_hooks" in sys.modules:
        return
    try:
        from antenv import axon_hooks  # noqa: F401
        return
    except ImportError:
        pass
    mod = types.ModuleType("antenv.axon_hooks")
    mod._hook = None
    mod.set_axon_ntff_profile_hook = lambda h: setattr(mod, "_hook", h)
    mod.get_axon_ntff_profile_hook = lambda: mod._hook
    sys.modules["antenv.axon_hooks"] = mod
    antenv.axon_hooks = mod


_ensure_axon_hooks()

B, S, D = 2, 1024, 4096
H, KVH, HD = 32, 8, 128
NCORES = 8
HPC = H // NCORES            # 4 q heads per core
QCOLS = HPC * HD             # 512
T = B * S                    # 2048
P = 128
KT = D // P                  # 32 k tiles
NQ = 4                       # token quarters (512 tokens each)
QW = T // NQ                 # 512
SQC = 2                      # sq chunks per batch
STB = S // P                 # 8 st blocks per batch
XTC = 4                      # k-tiles per xt DMA chunk
AGC = 2                      # k-tiles per allgather-readback DMA chunk

FP32 = mybir.dt.float32
BF16 = mybir.dt.bfloat16
EXP = mybir.ActivationFunctionType.Exp

_COMPILED = {}
LAST_RESULTS = None


def _st_info(variant, sqc):
    """st blocks contributing to sq chunk sqc, as (st, r, sel):
    r = first needed column within the 512-wide chunk (0 for full width),
    sel = start of the 128-wide diagonal span needing triangular zeroing
    (None if the block is fully below the diagonal / no mask)."""
    out = []
    for st in range(STB):
        if variant == "causal":
            rd = st * P - sqc * QW
            if rd >= QW:
                continue  # fully masked
            if rd >= 0:
                out.append((st, rd, rd))
            else:
                out.append((st, 0, None))
        else:
            out.append((st, 0, None))
    return out


def _build(variant):
    nc = bacc.Bacc(None)

    # xt packed quarter-major: [:, qx, k, :] is per-partition contiguous 4KB
    # per 4-ktile chunk, so xt chunk DMAs run at large-descriptor efficiency.
    xt_e = nc.declare_dram_parameter("xt", [P, NQ, KT, QW], BF16, isOutput=False)
    wq_e = nc.declare_dram_parameter("wq", [P, KT, QCOLS], BF16, isOutput=False)
    wk_e = nc.declare_dram_parameter("wk", [P, KT, HD], BF16, isOutput=False)
    wv_e = nc.declare_dram_parameter("wv", [P, KT, HD], BF16, isOutput=False)
    wo_e = nc.declare_dram_parameter("wo", [P, KT, QCOLS], BF16, isOutput=False)
    # cos: [c; c] duplicated halves.  sin: [s; -s] (negated bottom half).
    cos_e = nc.declare_dram_parameter("cos", [P, T], BF16, isOutput=False)
    sin_e = nc.declare_dram_parameter("sin", [P, T], BF16, isOutput=False)
    if variant == "general":
        mk_e = nc.declare_dram_parameter("mk", [P, STB, S], BF16, isOutput=False)
    out_e = nc.declare_dram_parameter("out", [QCOLS, T], BF16, isOutput=True)

    with tile.TileContext(nc) as tc:
        with (
            tc.tile_pool(name="wpool", bufs=1) as wpool,
            tc.tile_pool(name="cst", bufs=1) as cst,
            tc.tile_pool(name="persist", bufs=1) as persist,
            tc.tile_pool(name="xt", bufs=10) as xtp,
            tc.tile_pool(name="ev", bufs=4) as evp,
            tc.tile_pool(name="rt", bufs=3) as rtp,
            tc.tile_pool(name="probs", bufs=18) as prp,
            tc.tile_pool(name="misc", bufs=2) as mip,
            tc.tile_pool(name="ag", bufs=8) as agp,
            tc.tile_pool(name="ow", bufs=8) as owp,
            tc.tile_pool(name="ps", bufs=8, space="PSUM") as psp,
            tc.tile_pool(name="dram", bufs=1, space="DRAM") as dram,
        ):
            # ---- resident weights / constants ----
            # wq_sb doubles as wo storage: wo is DMA'd over it after proj
            # quarter 3's last wq read (Tile WAR tracking orders this).
            wq_sb = wpool.tile([P, KT, QCOLS], BF16, name="wq_sb")
            wk_sb = wpool.tile([P, KT, HD], BF16, name="wk_sb")
            wv_sb = wpool.tile([P, KT, HD], BF16, name="wv_sb")
            cos_sb = wpool.tile([P, T], BF16, name="cos_sb")
            sin_sb = wpool.tile([P, T], BF16, name="sin_sb")
            if variant == "general":
                mk_sb = wpool.tile([P, STB, S], BF16, name="mk_sb")

            ident = cst.tile([P, P], BF16, name="ident")
            make_identity(nc, ident)
            ones_sq = cst.tile([P, P], BF16, name="ones_sq")
            nc.vector.memset(ones_sq[:], 1.0)

            # ---- persistent activations ----
            q_rot = [[persist.tile([P, S], BF16, name=f"q{h}_{b}")
                      for b in range(B)] for h in range(HPC)]
            k_rot = [persist.tile([P, S], BF16, name=f"k{b}") for b in range(B)]
            v_sb = [persist.tile([P, STB, P], BF16, name=f"v{b}") for b in range(B)]
            attn = [[persist.tile([P, S], BF16, name=f"attn{h}_{b}")
                     for b in range(B)] for h in range(HPC)]

            # per-(batch, sq-half) gather buffers: two smaller collectives per
            # batch, each fired as soon as its attention chunk finishes --
            # spreads collective traffic and halves skew exposure.
            ag_in = [[dram.tile([HPC * P, QW], BF16, name=f"agin{b}_{c}")
                      for c in range(SQC)] for b in range(B)]
            ag_out = [[dram.tile([H * P, QW], BF16, addr_space="Shared",
                                 name=f"agout{b}_{c}") for c in range(SQC)]
                      for b in range(B)]

            def rope(dst, dst_off, src_bf, qoff):
                """RoPE on split layout (a=0:64, b=64:128).
                p1 = [a*c; b*c];  p2 = [a*s; -b*s];  swap halves of p2;
                dst = p1 + p2sw = [a*c - b*s; a*s + b*c]."""
                c = cos_sb[:, qoff:qoff + QW]
                s = sin_sb[:, qoff:qoff + QW]
                p1 = rtp.tile([P, QW], BF16, name="p1")
                p2 = rtp.tile([P, QW], BF16, name="p2")
                p2sw = rtp.tile([P, QW], BF16, name="p2sw")
                nc.vector.tensor_mul(p1[:], src_bf[:], c)
                nc.vector.tensor_mul(p2[:], src_bf[:], s)
                nc.vector.tensor_copy(p2sw[0:64, :], p2[64:128, :])
                nc.vector.tensor_copy(p2sw[64:128, :], p2[0:64, :])
                nc.vector.tensor_add(dst[:, dst_off:dst_off + QW], p1[:], p2sw[:])

            def load_weights():
                """Stream all projection weights + rope tables on the scalar
                (HWDGE) queue in big chunks, first-needed k-tiles first, so the
                sync queue only carries the xt stream.  The rope-table slices
                (needed only at each quarter's eviction, ~50us deadlines) are
                slotted mid-stream to keep them out of the startup burst."""
                dma = nc.scalar.dma_start

                def tables(qx):
                    toff = qx * QW
                    dma(cos_sb[:, toff:toff + QW], cos_e[:, toff:toff + QW])
                    dma(sin_sb[:, toff:toff + QW], sin_e[:, toff:toff + QW])

                # tiny first chunks so k=0 matmuls start ASAP
                dma(wk_sb[:, 0:2, :], wk_e[:, 0:2, :])
                dma(wv_sb[:, 0:2, :], wv_e[:, 0:2, :])
                dma(wq_sb[:, 0:2, :], wq_e[:, 0:2, :])
                dma(wk_sb[:, 2:8, :], wk_e[:, 2:8, :])
                dma(wv_sb[:, 2:8, :], wv_e[:, 2:8, :])
                for c in range(1, 5):
                    dma(wq_sb[:, 2 * c:2 * c + 2, :], wq_e[:, 2 * c:2 * c + 2, :])
                dma(wk_sb[:, 8:KT, :], wk_e[:, 8:KT, :])
                dma(wv_sb[:, 8:KT, :], wv_e[:, 8:KT, :])
                tables(0)
                for c in range(5, KT // 2):
                    dma(wq_sb[:, 2 * c:2 * c + 2, :], wq_e[:, 2 * c:2 * c + 2, :])
                    if c == 8:
                        tables(1)
                tables(2)
                tables(3)
                if variant == "general":
                    dma(mk_sb[:], mk_e[:])

            def wo_load():
                """Overwrite wq_sb with wo (WAR-ordered after the last wq read,
                i.e. streams during attention_batch(1)).  On the scalar queue so
                the sync queue only carries agt readback for wo_batch(0)."""
                for c in range(4):
                    nc.scalar.dma_start(wq_sb[:, 8 * c:8 * c + 8, :],
                                        wo_e[:, 8 * c:8 * c + 8, :])

            def proj_quarter(qx):
                b, boff = qx // 2, (qx % 2) * QW
                toff = qx * QW
                # psums: k, v, q0..q3
                psums = [psp.tile([P, QW], FP32, name="ps", tag="ps")
                         for _ in range(6)]

                def mm(mb, k, xt):
                    if mb == 0:
                        w = wk_sb[:, k, :]
                    elif mb == 1:
                        w = wv_sb[:, k, :]
                    else:
                        w = wq_sb[:, k, (mb - 2) * P:(mb - 1) * P]
                    nc.tensor.matmul(psums[mb][:], w, xt,
                                     start=(k == 0), stop=(k == KT - 1))

                # quarter 0 uses small leading chunks (first matmuls start
                # sooner during the startup HBM burst)
                chunks = ([1, 1, 2, 2, 2] + [XTC] * 6) if qx == 0 \
                    else [XTC] * (KT // XTC)
                k0 = 0
                tail = []
                for ci, cw in enumerate(chunks):
                    xt4 = xtp.tile([P, XTC, QW], BF16, name="xt4")
                    nc.sync.dma_start(xt4[:, 0:cw, :],
                                      xt_e[:, qx, k0:k0 + cw, :])
                    if ci < len(chunks) - 2:
                        for j in range(cw):
                            for mb in range(6):
                                mm(mb, k0 + j, xt4[:, j, :])
                    else:
                        tail += [(k0 + j, xt4[:, j, :]) for j in range(cw)]
                    k0 += cw
                # last two chunks mb-major, V then K then Q heads: each
                # accumulation stops several us early so its eviction +
                # rope/transpose overlaps the remaining matmuls and the
                # following attention chunk starts without a gap.
                for mb in (1, 0, 2, 3, 4, 5):
                    for k, xt in tail:
                        mm(mb, k, xt)
                # scalar queue order: ve, ke, qe0..3, v_sb copies — so the k/q
                # ropes (DVE) start as early as possible; the v_sb copies only
                # gate the PV matmuls which run several us into attention.
                ve = evp.tile([P, QW], BF16, name="ve", tag="qe")
                nc.scalar.copy(ve[:], psums[1][:])
                tps = []
                for i in range(QW // P):
                    tp = psp.tile([P, P], BF16, name="tp", tag="ps")
                    nc.tensor.transpose(tp[:], ve[:, i * P:(i + 1) * P], ident[:])
                    tps.append(tp)
                ke = evp.tile([P, QW], BF16, name="ke", tag="qe")
                nc.scalar.copy(ke[:], psums[0][:])
                rope(k_rot[b], boff, ke, toff)
                for h in range(HPC):
                    qe = evp.tile([P, QW], BF16, name="qe", tag="qe")
                    nc.scalar.copy(qe[:], psums[2 + h][:])
                    rope(q_rot[h][b], boff, qe, toff)
                for i, tp in enumerate(tps):
                    st = (qx % 2) * 4 + i
                    nc.scalar.copy(v_sb[b][:, st, :], tp[:])

            def attention_batch(b, sqcs=tuple(range(SQC))):
                for sqc in sqcs:
                    sq0 = sqc * QW
                    stl = _st_info(variant, sqc)
                    n = len(stl)
                    for h in range(HPC):
                        # scores^T + exp, narrowed to [r:QW] per st block
                        prtiles = []
                        for st, r, sel in stl:
                            pss = psp.tile([P, QW], FP32, name="pss", tag="ps")
                            preload = variant == "general"
                            if preload:
                                nc.tensor.matmul(pss[:], ident[:],
                                                 mk_sb[:, st, sq0:sq0 + QW],
                                                 start=True, stop=False)
                            nc.tensor.matmul(
                                pss[:, r:QW], k_rot[b][:, st * P:(st + 1) * P],
                                q_rot[h][b][:, sq0 + r:sq0 + QW],
                                start=(not preload), stop=True)
                            pr = prp.tile([P, QW], BF16, name="pr", tag="pr")
                            nc.scalar.activation(pr[:, r:QW], pss[:, r:QW], EXP)
                            if sel is not None and variant == "causal":
                                # zero probs in the diagonal 128-wide span where
                                # st*128+p > sq0+q (future keys); columns beyond
                                # sel+P are fully below the diagonal.
                                nc.gpsimd.affine_select(
                                    out=pr[:, sel:sel + P],
                                    in_=pr[:, sel:sel + P],
                                    compare_op=mybir.AluOpType.is_ge,
                                    fill=0.0,
                                    base=0,
                                    channel_multiplier=-1,
                                    pattern=[[1, P]])
                            prtiles.append((pr, r))
                        # PV and denominator (denom broadcast to all partitions)
                        pso = psp.tile([P, QW], FP32, name="pso", tag="ps")
                        psdb = psp.tile([P, QW], FP32, name="psdb", tag="ps")
                        for i, (st, r, sel) in enumerate(stl):
                            pr = prtiles[i][0]
                            nc.tensor.matmul(pso[:, r:QW], v_sb[b][:, st, :],
                                             pr[:, r:QW],
                                             start=(i == 0), stop=(i == n - 1))
                            nc.tensor.matmul(psdb[:, r:QW], ones_sq[:],
                                             pr[:, r:QW],
                                             start=(i == 0), stop=(i == n - 1))
                        rb = mip.tile([P, QW], FP32, name="rb")
                        nc.vector.reciprocal_approx_fast(rb[:], psdb[:])
                        nc.vector.tensor_mul(attn[h][b][:, sq0:sq0 + QW],
                                             pso[:], rb[:])
                        # ship this head's chunk to the gather bounce ASAP
                        nc.gpsimd.dma_start(
                            ag_in[b][sqc][h * P:(h + 1) * P, :],
                            attn[h][b][:, sq0:sq0 + QW])

            def gather_batch(b, c):
                nc.gpsimd.collective_compute(
                    "AllGather", mybir.AluOpType.bypass,
                    ins=[ag_in[b][c][:].opt()],
                    outs=[ag_out[b][c][:].opt()],
                    replica_groups=[list(range(NCORES))],
                )

            def wo_batch(b):
                ag_r = [ag_out[b][c].rearrange("(k p) t -> p k t", p=P)
                        for c in range(SQC)]
                psw = [[psp.tile([P, QW], FP32, name="psw", tag="ps")
                        for _ in range(SQC)] for _ in range(4)]
                nchk = KT // AGC
                for kc in range(nchk):
                    agt = [agp.tile([P, AGC, QW], BF16, name="agt")
                           for _ in range(SQC)]
                    for c in range(SQC):
                        nc.sync.dma_start(agt[c][:],
                                          ag_r[c][:, kc * AGC:(kc + 1) * AGC, :])
                    if kc < nchk - 1:
                        for j in range(AGC):
                            k = kc * AGC + j
                            for mb in range(4):
                                w = wq_sb[:, k, mb * P:(mb + 1) * P]
                                for nch in range(SQC):
                                    nc.tensor.matmul(
                                        psw[mb][nch][:], w, agt[nch][:, j, :],
                                        start=(k == 0), stop=False)
                    else:
                        # last chunk mb-major so early mb groups stop a few us
                        # before the end and their evictions + out DMAs overlap
                        # the remaining matmuls.
                        for mb in range(4):
                            for nch in range(SQC):
                                for j in range(AGC):
                                    k = kc * AGC + j
                                    w = wq_sb[:, k, mb * P:(mb + 1) * P]
                                    nc.tensor.matmul(
                                        psw[mb][nch][:], w, agt[nch][:, j, :],
                                        start=False, stop=(k == KT - 1))
                for mb in range(4):
                    for nch in range(SQC):
                        ow = owp.tile([P, QW], BF16, name="ow")
                        if (mb * SQC + nch) % 2 == 0:
                            nc.scalar.copy(ow[:], psw[mb][nch][:])
                            dma = nc.scalar.dma_start
                        else:
                            nc.vector.tensor_copy(ow[:], psw[mb][nch][:])
                            dma = nc.sync.dma_start
                        dma(out_e[mb * P:(mb + 1) * P,
                                  b * S + nch * QW:b * S + (nch + 1) * QW],
                            ow[:])

            # ---- timeline ----
            # attention sq-chunks interleave between proj quarters: chunk s0 of
            # batch b only needs that batch's first token quarter. This fires
            # gather(0) earlier and gives the xt/weight streams HBM-quiet
            # windows (attention phases do no HBM traffic) to get ahead.
            load_weights()
            proj_quarter(0)
            attention_batch(0, (0,))
            gather_batch(0, 0)
            proj_quarter(1)
            attention_batch(0, (1,))
            gather_batch(0, 1)
            proj_quarter(2)
            attention_batch(1, (0,))
            gather_batch(1, 0)
            proj_quarter(3)
            wo_load()
            attention_batch(1, (1,))
            gather_batch(1, 1)
            wo_batch(0)
            wo_batch(1)

    nc.compile()
    return nc


def _get_compiled(variant):
    if variant not in _COMPILED:
        _COMPILED[variant] = _build(variant)
    return _COMPILED[variant]


def _detect_variant(mask2d):
    if not np.any(mask2d):
        return "nomask"
    tril = np.tril(mask2d)
    if not np.any(tril):
        iu = np.triu_indices(S, 1)
        if np.all(mask2d[iu] <= -1e8):
            return "causal"
    return "general"


def _pack_kt(w):
    """[R*128, N] -> [128, R, N] so that [:, k, :] is rows k*128..k*128+127."""
    return np.ascontiguousarray(w.reshape(w.shape[0] // P, P, -1).transpose(1, 0, 2))


def kernel(x, wq, wk, wv, wo, lora_q_a, lora_q_b, lora_v_a, lora_v_b,
           freqs_cos, freqs_sin, mask, start_pos=0, **_):
    global LAST_RESULTS
    bf = ml_dtypes.bfloat16
    x = np.asarray(x, np.float32)
    wq = np.asarray(wq, np.float32)
    wk = np.asarray(wk, np.float32)
    wv = np.asarray(wv, np.float32)
    wo = np.asarray(wo, np.float32)
    lora_q_a = np.asarray(lora_q_a, np.float32)
    lora_q_b = np.asarray(lora_q_b, np.float32)
    lora_v_a = np.asarray(lora_v_a, np.float32)
    lora_v_b = np.asarray(lora_v_b, np.float32)
    cos = np.asarray(freqs_cos, np.float32)
    sin = np.asarray(freqs_sin, np.float32)
    mask2d = np.asarray(mask, np.float32).reshape(S, S)

    variant = _detect_variant(mask2d)
    nc = _get_compiled(variant)

    # fold LoRA + scale; permute rope pairs (evens then odds within each head)
    wq_eff = (wq + lora_q_a @ lora_q_b) * np.float32(1.0 / np.sqrt(HD))
    wv_eff = wv + lora_v_a @ lora_v_b
    perm = np.concatenate([np.arange(0, HD, 2), np.arange(1, HD, 2)])
    qperm = (np.arange(H)[:, None] * HD + perm[None, :]).reshape(-1)
    kperm = (np.arange(KVH)[:, None] * HD + perm[None, :]).reshape(-1)
    wq_eff = wq_eff[:, qperm]
    wk_p = wk[:, kperm]

    xt = np.ascontiguousarray(x.reshape(T, D).T)        # [4096, 2048]
    # [128, KT, T] -> quarter-major [128, NQ, KT, QW] (contiguous per chunk)
    xt_p = np.ascontiguousarray(
        _pack_kt(xt).reshape(P, KT, NQ, QW).transpose(0, 2, 1, 3)).astype(bf)
    c64 = np.tile(cos.T, (1, B))                        # [64, 2048]
    s64 = np.tile(sin.T, (1, B))
    cosT = np.concatenate([c64, c64], axis=0).astype(bf)   # [c; c]
    sinT = np.concatenate([s64, -s64], axis=0).astype(bf)  # [s; -s]

    if variant == "general":
        maskT = np.ascontiguousarray(mask2d.T)          # [st, sq]
        mk = _pack_kt(maskT).astype(bf)                 # [128, 8, 1024]
    else:
        mk = None

    in_maps = []
    for c in range(NCORES):
        im = {
            "xt": xt_p,
            "wq": _pack_kt(wq_eff[:, c * QCOLS:(c + 1) * QCOLS]).astype(bf),
            "wk": _pack_kt(wk_p[:, c * HD:(c + 1) * HD]).astype(bf),
            "wv": _pack_kt(wv_eff[:, c * HD:(c + 1) * HD]).astype(bf),
            "wo": _pack_kt(wo[:, c * QCOLS:(c + 1) * QCOLS]).astype(bf),
            "cos": cosT,
            "sin": sinT,
        }
        if mk is not None:
            im["mk"] = mk
        in_maps.append(im)

    res = run_bass_kernel_spmd(nc, in_maps, core_ids=list(range(NCORES)))
    LAST_RESULTS = res
    outT = np.concatenate([res.results[c]["out"] for c in range(NCORES)], axis=0)
    return np.ascontiguousarray(outT.T).reshape(B, S, D).astype(np.float32)
